# revision 10
# baseline (speedup 1.0000x reference)
"""Trainium2 Bass kernel for MultiHeadLinearAttentionLayer (v2).

Problem (hardcoded shapes): B=4, L=S=2048, D_MODEL=1024, N_HEADS=16, HEAD_DIM=64.
  q/k/v = x @ W + b; RoPE(q), RoPE(k); qf/kf = elu(.)+1; kf masked by key_lengths;
  kv = kf^T v, ksum = sum kf; out = (qf @ kv) / (qf @ ksum + eps); y = out @ Wo + bo.

Sharding: 8 cores = 4 batches x 2 head-groups (8 heads each). Each core computes a
partial y (its head-group's contribution through Wo rows); host sums the two
partials per batch and adds bo. All matmuls bf16 (fp32 PSUM accumulation).

v2 design notes (engine-balance rework of v1):
  - Elementwise chains rebuilt around the DVE perf-mode rules: all-bf16 packed
    SBUF tensor_tensor runs 2x, tensor_scalar/copy 4x, while any PSUM/f32
    operand drops to 1x.  PSUM evacuation is done on the Scalar engine; all
    subsequent vector work is bf16 SBUF.
  - Q RoPE uses q2 = qt*cos + R@(qt*sin)  (valid because paired rows share the
    same sin row), so the rotate matmul input is an SBUF bf16 tile and no
    extra PSUM-read multiply exists.
  - elu(x)+1 = max(x,0) + exp(min(x,0)): min on DVE (4x), exp on Scalar, and
    the relu+add fused into ONE gpsimd scalar_tensor_tensor.
  - Q/K phases process 1024-wide halfgroups (two 512 PSUM banks evacuated into
    one [128,1024] tile) to halve op counts; Q's k-outer/c-inner matmul order
    shares each LDWEIGHTS across 2 matmuls.
  - V phase folds the key-length mask into the Scalar PSUM-evac via a
    per-partition scale AP (token-major: mask is per-partition).
  - attn tail: osb = opp(PSUM) * zep(PSUM) directly on DVE (no zes evac); y is
    stored bf16 with one [128,1024] store per token tile.
  - K-phase cos/sin tables shrunk 16x ([128,NT*64] + head-broadcast APs).
  - DMA issue spread across sync/scalar/vector/gpsimd queues at startup
    (each dma_start costs ~0.6us of issuing-queue time).
"""

import numpy as np
import ml_dtypes

import os
import concourse.bacc as bacc
import concourse.mybir as mybir
from concourse import tile
from concourse.bass_utils import run_bass_kernel_spmd

BF16 = mybir.dt.bfloat16
F32 = mybir.dt.float32
AF = mybir.ActivationFunctionType
ALU = mybir.AluOpType
BF = ml_dtypes.bfloat16

D_MODEL = 1024
N_HEADS = 16
HEAD_DIM = 64
ROPE_THETA = 10000.0
T = 2048          # L = S
NT = T // 128     # 16 token tiles
NC_ = 4           # token chunks of 512
NK = D_MODEL // 128   # 8 contraction tiles
DQ = 512          # per-core head dims (8 heads x 64)
NJ = DQ // 128    # 4 dq tiles
NH = 8            # heads per core

LAST_RESULTS = None  # stashed BassKernelResults for test harnesses


def _build_program(with_bq, with_bk, with_bv):
    nc = bacc.Bacc("TRN2", target_bir_lowering=False)
    MULTIQ = int(os.environ.get("KNOB_DMAQ", "1"))

    def _dq(eng_of_nc):
        return eng_of_nc if MULTIQ else nc.sync

    xqt_d = nc.declare_dram_parameter("xqt", [D_MODEL, T], BF16, isOutput=False)
    xkt_d = nc.declare_dram_parameter("xkt", [D_MODEL, T], BF16, isOutput=False)
    xvt_d = nc.declare_dram_parameter("xvt", [D_MODEL, T], BF16, isOutput=False)
    wq = nc.declare_dram_parameter("wq", [D_MODEL, DQ], BF16, isOutput=False)
    wk = nc.declare_dram_parameter("wk", [D_MODEL, DQ], BF16, isOutput=False)
    wv = nc.declare_dram_parameter("wv", [D_MODEL, DQ], BF16, isOutput=False)
    wo = nc.declare_dram_parameter("wo", [DQ, D_MODEL], BF16, isOutput=False)
    cosfm = nc.declare_dram_parameter("cosfm", [128, T], BF16, isOutput=False)
    sinfm = nc.declare_dram_parameter("sinfm", [128, T], BF16, isOutput=False)
    costsm = nc.declare_dram_parameter("costsm", [128, NT * 64], BF16, isOutput=False)
    sintsm = nc.declare_dram_parameter("sintsm", [128, NT * 64], BF16, isOutput=False)
    rt = nc.declare_dram_parameter("rt", [128, 128], BF16, isOutput=False)
    eselp = nc.declare_dram_parameter("esel", [128, 128], BF16, isOutput=False)
    maskc = nc.declare_dram_parameter("maskc", [128, NT], F32, isOutput=False)
    bq = nc.declare_dram_parameter("bq", [1, DQ], BF16, isOutput=False) if with_bq else None
    bk = nc.declare_dram_parameter("bk", [1, DQ], BF16, isOutput=False) if with_bk else None
    bv = nc.declare_dram_parameter("bv", [1, DQ], BF16, isOutput=False) if with_bv else None
    y = nc.declare_dram_parameter("y", [T, D_MODEL], BF16, isOutput=True)

    with tile.TileContext(nc) as tc:
        with tc.tile_pool(name="sb", bufs=1) as sb, \
             tc.tile_pool(name="ps", bufs=1, space="PSUM") as ps:

            def tmp(name):
                return sb.tile([128, 1024], BF16, tag="tmp", bufs=14, name=name)

            # ---- constants / weights / Q inputs -------------------------
            # sync queue: wq k-tiles interleaved with xq low halves (critical
            # path for the first halfgroup); scalar queue: Q tables; gpsimd
            # queue: xq high halves.
            wq_sb = sb.tile([128, NK, DQ], BF16, tag="w", bufs=3)
            xq = [sb.tile([128, T], BF16, tag="xt", bufs=16, name=f"xq{k}")
                  for k in range(NK)]
            for k in range(NK):
                nc.sync.dma_start(wq_sb[:, k, :], wq[128 * k:128 * (k + 1), :])
                nc.sync.dma_start(xq[k][:, 0:1024],
                                  xqt_d[128 * k:128 * (k + 1), 0:1024])
            sinf = sb.tile([128, T], BF16, tag="fm", bufs=2, name="sinf")
            _dq(nc.scalar).dma_start(sinf[:], sinfm[:])
            cosf = sb.tile([128, T], BF16, tag="fm", bufs=2, name="cosf")
            _dq(nc.scalar).dma_start(cosf[:], cosfm[:])
            rt_sb = sb.tile([128, 128], BF16, tag="rt")
            _dq(nc.scalar).dma_start(rt_sb[:], rt[:])
            for k in range(NK):
                _dq(nc.gpsimd).dma_start(xq[k][:, 1024:2048],
                                    xqt_d[128 * k:128 * (k + 1), 1024:2048])
            ones = sb.tile([1, 512], BF16, tag="ones")
            nc.vector.memset(ones[:], 1.0)
            zrow = sb.tile([1, 512], BF16, tag="zrow")
            nc.vector.memset(zrow[:], 0.0)
            if with_bq:
                bq_sb = sb.tile([1, DQ], BF16, tag="brow", bufs=3)
                nc.sync.dma_start(bq_sb[:], bq[:])

            qf_all = sb.tile([128, NJ, T], BF16, tag="qf")

            # ---- Q phase: 8 halfgroups (cp, j), 1024 tokens each --------
            NQG = 2 * NJ
            qst = [dict() for _ in range(NQG)]

            def q_A(i):      # 16 proj matmuls, k-outer/c-inner (LDW shared x2)
                cp, j = divmod(i, NJ)
                ps0 = ps.tile([128, 512], F32, tag="mm", bufs=5, name="psq0")
                ps1 = ps.tile([128, 512], F32, tag="mm", bufs=5, name="psq1")
                pss = (ps0, ps1)
                first = (True, True)
                if with_bq:
                    for ci in range(2):
                        nc.tensor.matmul(pss[ci][:],
                                         bq_sb[:, 128 * j:128 * (j + 1)],
                                         ones[:], start=True, stop=False)
                    first = (False, False)
                for k in range(NK):
                    for ci in range(2):
                        c = 2 * cp + ci
                        nc.tensor.matmul(
                            pss[ci][:], wq_sb[:, k, 128 * j:128 * (j + 1)],
                            xq[k][:, 512 * c:512 * (c + 1)],
                            start=(k == 0 and first[ci]), stop=(k == NK - 1))
                qst[i]["pss"] = pss

            def q_B(i):      # scalar evac of both banks -> qt [128,1024]
                ps0, ps1 = qst[i]["pss"]
                qt = tmp("qt")
                nc.scalar.copy(qt[:, 0:512], ps0[:])
                nc.scalar.copy(qt[:, 512:1024], ps1[:])
                qst[i]["qt"] = qt

            def q_C(i):      # ts = qt*sin (V 2x); rot = R@ts (PE); t1 = qt*cos
                cp, j = divmod(i, NJ)
                ch = slice(1024 * cp, 1024 * (cp + 1))
                qt = qst[i]["qt"]
                ts = tmp("ts")
                nc.vector.tensor_tensor(ts[:], qt[:], sinf[:, ch], ALU.mult)
                rot0 = ps.tile([128, 512], F32, tag="aux", bufs=2, name="rot0")
                rot1 = ps.tile([128, 512], F32, tag="aux", bufs=2, name="rot1")
                nc.tensor.matmul(rot0[:], rt_sb[:], ts[:, 0:512],
                                 start=True, stop=True)
                nc.tensor.matmul(rot1[:], rt_sb[:], ts[:, 512:1024],
                                 start=True, stop=True)
                t1 = tmp("t1")
                nc.vector.tensor_tensor(t1[:], qt[:], cosf[:, ch], ALU.mult)
                qst[i]["rot"] = (rot0, rot1)
                qst[i]["t1"] = t1

            def q_D(i):      # q2 halves (V 1x PSUM); min/relu (V 4x)
                rot0, rot1 = qst[i]["rot"]
                t1 = qst[i]["t1"]
                q2 = tmp("q2")
                nc.vector.tensor_tensor(q2[:, 0:512], t1[:, 0:512], rot0[:],
                                        ALU.add)
                nc.vector.tensor_tensor(q2[:, 512:1024], t1[:, 512:1024],
                                        rot1[:], ALU.add)
                q2m = tmp("q2m")
                nc.vector.tensor_scalar_min(q2m[:], q2[:], 0.0)
                qr = tmp("qr")
                nc.vector.tensor_scalar_max(qr[:], q2[:], 0.0)
                qst[i]["q2m"] = q2m
                qst[i]["qr"] = qr

            def q_E(i):      # qe = exp(q2m) (S)
                qe = tmp("qe")
                nc.scalar.activation(qe[:], qst[i]["q2m"][:], AF.Exp)
                qst[i]["qe"] = qe

            def q_F(i):      # qf = relu + exp(min)  (gpsimd add)
                cp, j = divmod(i, NJ)
                ch = slice(1024 * cp, 1024 * (cp + 1))
                nc.gpsimd.tensor_tensor(
                    qf_all[:, j, ch], qst[i]["qr"][:], qst[i]["qe"][:], ALU.add)
                qst[i].clear()

            with nc.named_scope("qproj"):
                for s in range(NQG + 4):
                    if s < NQG:
                        q_A(s)
                    if 0 <= s - 1 < NQG:
                        q_B(s - 1)
                    if 0 <= s - 2 < NQG:
                        q_D(s - 2)
                    if 0 <= s - 1 < NQG:
                        q_C(s - 1)
                    if 0 <= s - 3 < NQG:
                        q_E(s - 3)
                    if 0 <= s - 4 < NQG:
                        q_F(s - 4)

            # ---- K phase: 8 halfgroups of 2 token tiles -----------------
            wk_sb = sb.tile([128, NK, DQ], BF16, tag="w", bufs=3)
            xk = [sb.tile([128, T], BF16, tag="xt", bufs=16, name=f"xk{k}")
                  for k in range(NK)]
            for k in range(NK):
                nc.sync.dma_start(wk_sb[:, k, :], wk[128 * k:128 * (k + 1), :])
                nc.sync.dma_start(xk[k][:, 0:1024],
                                  xkt_d[128 * k:128 * (k + 1), 0:1024])
                nc.sync.dma_start(xk[k][:, 1024:2048],
                                  xkt_d[128 * k:128 * (k + 1), 1024:2048])
            cost_sb = sb.tile([128, NT, 64], BF16, tag="tsm", bufs=2)
            sint_sb = sb.tile([128, NT, 64], BF16, tag="tsm", bufs=2)
            _dq(nc.gpsimd).dma_start(cost_sb[:],
                                costsm.rearrange("p (m c) -> p m c", m=NT))
            _dq(nc.gpsimd).dma_start(sint_sb[:],
                                sintsm.rearrange("p (m c) -> p m c", m=NT))
            if with_bk:
                bk_sb = sb.tile([1, DQ], BF16, tag="brow", bufs=3)
                nc.sync.dma_start(bk_sb[:], bk[:])

            kf_all = sb.tile([128, NT, DQ], BF16, tag="kf")
            NKG = NT // 2
            kst = [dict() for _ in range(NKG)]

            def k_A(g):      # 16 proj matmuls for token tiles 2g, 2g+1
                ps0 = ps.tile([128, 512], F32, tag="mm", bufs=5, name="psk0")
                ps1 = ps.tile([128, 512], F32, tag="mm", bufs=5, name="psk1")
                pss = (ps0, ps1)
                first = [True, True]
                if with_bk:
                    for mi in range(2):
                        nc.tensor.matmul(pss[mi][:], ones[:, 0:128], bk_sb[:],
                                         start=True, stop=False)
                        first[mi] = False
                for k in range(NK):
                    for mi in range(2):
                        m = 2 * g + mi
                        nc.tensor.matmul(
                            pss[mi][:], xk[k][:, 128 * m:128 * (m + 1)],
                            wk_sb[:, k, :],
                            start=(k == 0 and first[mi]), stop=(k == NK - 1))
                kst[g]["pss"] = pss

            def k_B(g):      # scalar evac -> kp [128, 2, 512]
                ps0, ps1 = kst[g]["pss"]
                kp = tmp("kp")
                kpv = kp.rearrange("p (m c) -> p m c", m=2)
                nc.scalar.copy(kpv[:, 0, :], ps0[:])
                nc.scalar.copy(kpv[:, 1, :], ps1[:])
                kst[g]["kp"] = kp

            def k_C(g):      # t1 = kp*cos (V 2x, head-bcast); t2 half-swaps
                kp = kst[g]["kp"]
                t1 = tmp("t1k")
                t1v = t1.rearrange("p (m h i) -> p m h i", m=2, h=NH)
                kpv = kp.rearrange("p (m h i) -> p m h i", m=2, h=NH)
                cbc = cost_sb[:, 2 * g:2 * g + 2, :] \
                    .rearrange("p m (a i) -> p m a i", a=1) \
                    .broadcast_to([128, 2, NH, 64])
                nc.vector.tensor_tensor(t1v[:], kpv[:], cbc[:], ALU.mult)
                t2 = tmp("t2k")
                for mi in range(2):
                    m = 2 * g + mi
                    k8 = kp[:, 512 * mi:512 * (mi + 1)] \
                        .rearrange("p (h s i) -> p h s i", h=NH, s=2, i=32)
                    t8 = t2[:, 512 * mi:512 * (mi + 1)] \
                        .rearrange("p (h s i) -> p h s i", h=NH, s=2, i=32)
                    sa = sint_sb[:, m, 0:32] \
                        .rearrange("p (a i) -> p a i", a=1) \
                        .broadcast_to([128, NH, 32])
                    sb_ = sint_sb[:, m, 32:64] \
                        .rearrange("p (a i) -> p a i", a=1) \
                        .broadcast_to([128, NH, 32])
                    nc.vector.tensor_tensor(t8[:, :, 0, :], k8[:, :, 1, :],
                                            sa[:], ALU.mult)
                    nc.vector.tensor_tensor(t8[:, :, 1, :], k8[:, :, 0, :],
                                            sb_[:], ALU.mult)
                kst[g]["t1"] = t1
                kst[g]["t2"] = t2

            def k_D(g):      # k2 = t1 + t2 (V 2x); min/relu (V 4x)
                k2 = tmp("k2")
                nc.vector.tensor_tensor(k2[:], kst[g]["t1"][:], kst[g]["t2"][:],
                                        ALU.add)
                k2m = tmp("k2m")
                nc.vector.tensor_scalar_min(k2m[:], k2[:], 0.0)
                kr = tmp("kr")
                nc.vector.tensor_scalar_max(kr[:], k2[:], 0.0)
                kst[g]["k2m"] = k2m
                kst[g]["kr"] = kr

            def k_E(g):      # ke = exp(k2m) (S)
                ke = tmp("ke")
                nc.scalar.activation(ke[:], kst[g]["k2m"][:], AF.Exp)
                kst[g]["ke"] = ke

            def k_F(g):      # kf = relu + exp(min)  (gpsimd add)
                kfv = kf_all[:, 2 * g:2 * g + 2, :]
                krv = kst[g]["kr"].rearrange("p (m c) -> p m c", m=2)
                kev = kst[g]["ke"].rearrange("p (m c) -> p m c", m=2)
                nc.gpsimd.tensor_tensor(kfv[:], krv[:], kev[:], ALU.add)
                kst[g].clear()

            with nc.named_scope("kproj"):
                for s in range(NKG + 4):
                    if s < NKG:
                        k_A(s)
                    if 0 <= s - 1 < NKG:
                        k_B(s - 1)
                    if 0 <= s - 2 < NKG:
                        k_D(s - 2)
                    if 0 <= s - 1 < NKG:
                        k_C(s - 1)
                    if 0 <= s - 3 < NKG:
                        k_E(s - 3)
                    if 0 <= s - 4 < NKG:
                        k_F(s - 4)

            # ---- V phase + kv accumulation ------------------------------
            wv_sb = sb.tile([128, NK, DQ], BF16, tag="w", bufs=3)
            mk_sb = sb.tile([128, NT], F32, tag="mask")
            nc.sync.dma_start(mk_sb[:], maskc[:])
            wo_sb = sb.tile([128, NJ, D_MODEL], BF16, tag="wo")
            esel = sb.tile([128, 128], BF16, tag="esel")
            xv = [sb.tile([128, T], BF16, tag="xt", bufs=16, name=f"xv{k}")
                  for k in range(NK)]
            nc.sync.dma_start(wv_sb[:], wv.rearrange("(k p) c -> p k c", p=128))
            for k in range(NK):
                nc.sync.dma_start(xv[k][:], xvt_d[128 * k:128 * (k + 1), :])
            nc.sync.dma_start(wo_sb[:], wo.rearrange("(k p) c -> p k c", p=128))
            nc.sync.dma_start(esel[:], eselp[:])
            if with_bv:
                bv_sb = sb.tile([1, DQ], BF16, tag="brow", bufs=3)
                nc.sync.dma_start(bv_sb[:], bv[:])

            kvp_t = ps.tile([128, 512], F32, tag="kv", bufs=1, name="kvp")
            kvp = kvp_t[:, 0:272]
            # open the kv accumulation group: zero the whole region so later
            # disjoint-region matmuls (start=False) all accumulate onto it
            nc.tensor.matmul(kvp[:], zrow[:, 0:128], zrow[:, 0:272],
                             start=True, stop=False)
            vst = [dict() for _ in range(NT)]

            def v_A(m):
                psv = ps.tile([128, 512], F32, tag="mm", bufs=5, name="psv")
                first = True
                if with_bv:
                    nc.tensor.matmul(psv[:], ones[:, 0:128], bv_sb[:],
                                     start=True, stop=False)
                    first = False
                for k in range(NK):
                    nc.tensor.matmul(
                        psv[:], xv[k][:, 128 * m:128 * (m + 1)],
                        wv_sb[:, k, :], start=first, stop=(k == NK - 1))
                    first = False
                vst[m]["psv"] = psv

            def v_B(m):      # scalar evac with fused mask scale; V mask col
                psv = vst[m]["psv"]
                v2 = sb.tile([128, NH, 68], BF16, tag="vv", bufs=4, name="v2")
                nc.scalar.activation(
                    v2[:, :, 0:64], psv.rearrange("p (h i) -> p h i", h=NH),
                    AF.Copy, scale=mk_sb[:, m:m + 1])
                nc.vector.tensor_copy(
                    v2[:, :, 64:65],
                    mk_sb[:, m:m + 1].rearrange("p (a i) -> p a i", a=1)
                    .broadcast_to([128, NH, 1]))
                vst[m]["v2"] = v2

            def v_C(m):      # kv' accumulation
                v2 = vst[m]["v2"]
                for h in range(NH):
                    r0 = 64 * (h % 2)
                    c0 = 68 * (h // 2)
                    nc.tensor.matmul(
                        kvp[r0:r0 + 64, c0:c0 + 68],
                        kf_all[:, m, 64 * h:64 * (h + 1)],
                        v2[:, h, :],
                        start=False, stop=False,
                        tile_position=(0, r0))
                vst[m].clear()

            with nc.named_scope("vproj"):
                for s in range(NT + 1):
                    if s < NT:
                        v_A(s)
                    if 0 <= s - 1 < NT:
                        v_B(s - 1)
                        v_C(s - 1)
            # close the kv group (single dep covering all kv matmuls)
            nc.tensor.matmul(kvp[:], zrow[:, 0:128], zrow[:, 0:272],
                             start=False, stop=True)

            # repack kv' into per-j block-diagonal [128,128] + ksum pack [128,8]
            kvblk = [sb.tile([128, 128], BF16, tag="kvb", bufs=NJ, name=f"kvb{j}")
                     for j in range(NJ)]
            kspack = sb.tile([128, 8], BF16, tag="ksp")
            nc.vector.memset(kspack[:], 0.0)
            for j in range(NJ):
                nc.vector.memset(kvblk[j][:], 0.0)
                nc.vector.tensor_copy(kvblk[j][0:64, 0:64],
                                      kvp[0:64, 68 * j:68 * j + 64])
                nc.vector.tensor_copy(kvblk[j][64:128, 64:128],
                                      kvp[64:128, 68 * j:68 * j + 64])
                nc.vector.tensor_copy(kspack[0:64, 2 * j:2 * j + 1],
                                      kvp[0:64, 68 * j + 64:68 * j + 65])
                nc.vector.tensor_copy(kspack[64:128, 2 * j + 1:2 * j + 2],
                                      kvp[64:128, 68 * j + 64:68 * j + 65])

            # ---- attention (feature-major) + output projection ----------
            ast = [dict() for _ in range(NC_)]

            def a_den(c):
                ch = slice(512 * c, 512 * (c + 1))
                den = ps.tile([128, 512], F32, tag="aux", bufs=2, name="den")
                for j in range(NJ):
                    nc.tensor.matmul(den[32 * j:32 * j + 2, :],
                                     kspack[:, 2 * j:2 * j + 2],
                                     qf_all[:, j, ch], start=True, stop=True,
                                     tile_position=(0, 32 * j))
                zrecf = sb.tile([128, 512], F32, tag="zrf", bufs=2, name="zrecf")
                with nc.allow_low_precision(reason="z scale approx is fine"):
                    nc.vector.reciprocal_approx_fast(zrecf[:], den[:])
                zrec = sb.tile([128, 512], BF16, tag="zr", bufs=2, name="zrec")
                nc.vector.tensor_copy(zrec[:], zrecf[:])
                ast[c]["zrec"] = zrec

            def a_attn(c):
                ch = slice(512 * c, 512 * (c + 1))
                zrec = ast[c]["zrec"]
                osbs = []
                for j in range(NJ):
                    zep = ps.tile([128, 512], F32, tag="mm", bufs=5, name="zep")
                    nc.tensor.matmul(zep[:], esel[32 * j:32 * j + 2, :],
                                     zrec[32 * j:32 * j + 2, :],
                                     start=True, stop=True,
                                     tile_position=(32 * j, 0))
                    zes = sb.tile([128, 512], BF16, tag="ze", bufs=4,
                                  name="zes")
                    nc.scalar.copy(zes[:], zep[:])
                    opp = ps.tile([128, 512], F32, tag="mm", bufs=5, name="opp")
                    nc.tensor.matmul(opp[:], kvblk[j][:], qf_all[:, j, ch],
                                     start=True, stop=True)
                    osb = sb.tile([128, 512], BF16, tag="osb", bufs=8,
                                  name=f"osb{j}")
                    nc.vector.tensor_tensor(osb[:], opp[:], zes[:], ALU.mult)
                    osbs.append(osb)
                ast[c]["osbs"] = osbs

            def a_y(c):
                osbs = ast[c]["osbs"]
                for mm_ in range(4):
                    m = 4 * c + mm_
                    yps = [ps.tile([128, 512], F32, tag="mm", bufs=5, name="yp")
                           for _ in range(2)]
                    for j in range(NJ):
                        for c2 in range(2):
                            nc.tensor.matmul(
                                yps[c2][:],
                                osbs[j][:, 128 * mm_:128 * (mm_ + 1)],
                                wo_sb[:, j, 512 * c2:512 * (c2 + 1)],
                                start=(j == 0), stop=(j == NJ - 1))
                    ysb = sb.tile([128, 1024], BF16, tag="ysb", bufs=4,
                                  name="ysb")
                    nc.scalar.copy(ysb[:, 0:512], yps[0][:])
                    if mm_ == 3:
                        nc.vector.tensor_copy(ysb[:, 512:1024], yps[1][:])
                    else:
                        nc.scalar.copy(ysb[:, 512:1024], yps[1][:])
                    nc.sync.dma_start(y[128 * m:128 * (m + 1), :], ysb[:])
                ast[c].clear()

            with nc.named_scope("attn"):
                for s in range(NC_ + 1):
                    if s < NC_:
                        a_den(s)
                    if 0 <= s - 1 < NC_:
                        a_attn(s - 1)
                        a_y(s - 1)

    nc.compile()
    return nc


def _host_prep(queries, keys, values, key_lengths, Wq, bq, Wk, bk, Wv, bv, Wo):
    """Build the per-core input maps (host side: transpose, cast, tables)."""
    B = queries.shape[0]
    # per-head [evens|odds] feature permutation
    pat = np.concatenate([np.arange(0, HEAD_DIM, 2), np.arange(1, HEAD_DIM, 2)])
    perm = np.concatenate([h * HEAD_DIM + pat for h in range(NH)])  # within 512

    inv_freq = 1.0 / (ROPE_THETA ** (np.arange(0, HEAD_DIM, 2, dtype=np.float64)
                                     / HEAD_DIM))  # [32]
    t = np.arange(T, dtype=np.float64)
    ang = t[:, None] * inv_freq[None, :]           # [T, 32]
    cos32 = np.cos(ang).astype(np.float32)
    sin32 = np.sin(ang).astype(np.float32)

    # feature-major tables [128, T]: row r: block = r % 64; i = block % 32
    idx = np.arange(128) % HEAD_DIM
    fidx = np.where(idx < 32, idx, idx - 32)
    cosfm = cos32[:, fidx].T.astype(BF)            # [128, T]
    sinfm = sin32[:, fidx].T.astype(BF)

    # token-major small tables [128, NT, 64]:
    #   costsm[p, m, i] = cos32[m*128+p, i mod 32]
    #   sintsm[p, m, 0:32] = -sin32[tok, :], [32:64] = +sin32[tok, :]
    cos_t = cos32.reshape(NT, 128, 32).transpose(1, 0, 2)    # [128, NT, 32]
    sin_t = sin32.reshape(NT, 128, 32).transpose(1, 0, 2)
    costsm = np.concatenate([cos_t, cos_t], axis=2)          # [128, NT, 64]
    sintsm = np.concatenate([-sin_t, sin_t], axis=2)
    costsm = np.ascontiguousarray(costsm.reshape(128, NT * 64)).astype(BF)
    sintsm = np.ascontiguousarray(sintsm.reshape(128, NT * 64)).astype(BF)

    # rotation matrix R (permuted layout), lhsT = R.T
    R = np.zeros((128, 128), np.float32)
    for base in (0, 64):
        R[base + 0:base + 32, base + 32:base + 64] = -np.eye(32)
        R[base + 32:base + 64, base + 0:base + 32] = np.eye(32)
    rt = np.ascontiguousarray(R.T).astype(BF)
    esel = np.zeros((128, 128), np.float32)
    for j in range(NJ):
        esel[32 * j, 0:64] = 1.0
        esel[32 * j + 1, 64:128] = 1.0
    esel = esel.astype(BF)

    with_bq = bool(np.any(np.asarray(bq)))
    with_bk = bool(np.any(np.asarray(bk)))
    with_bv = bool(np.any(np.asarray(bv)))

    xqt_b = [np.ascontiguousarray(np.asarray(queries[b]).astype(BF).T)
             for b in range(B)]
    xkt_b = [np.ascontiguousarray(np.asarray(keys[b]).astype(BF).T)
             for b in range(B)]
    xvt_b = [np.ascontiguousarray(np.asarray(values[b]).astype(BF).T)
             for b in range(B)]
    Wq = np.asarray(Wq, np.float32); Wk = np.asarray(Wk, np.float32)
    Wv = np.asarray(Wv, np.float32); Wo = np.asarray(Wo, np.float32)
    bq = np.asarray(bq, np.float32); bk = np.asarray(bk, np.float32)
    bv = np.asarray(bv, np.float32)

    in_maps = []
    for core in range(2 * B):
        b, g = core // 2, core % 2
        sl = slice(DQ * g, DQ * (g + 1))
        mask = (np.arange(T) < int(key_lengths[b])).astype(np.float32)
        maskc = np.ascontiguousarray(mask.reshape(NT, 128).T)
        m = {
            "xqt": xqt_b[b], "xkt": xkt_b[b], "xvt": xvt_b[b],
            "wq": np.ascontiguousarray(Wq[:, sl][:, perm]).astype(BF),
            "wk": np.ascontiguousarray(Wk[:, sl][:, perm]).astype(BF),
            "wv": np.ascontiguousarray(Wv[:, sl]).astype(BF),
            "wo": np.ascontiguousarray(Wo[sl, :]).astype(BF),
            "cosfm": cosfm, "sinfm": sinfm,
            "costsm": costsm, "sintsm": sintsm,
            "rt": rt, "esel": esel, "maskc": maskc,
        }
        if with_bq:
            m["bq"] = bq[sl][perm].reshape(1, DQ).astype(BF)
        if with_bk:
            m["bk"] = bk[sl][perm].reshape(1, DQ).astype(BF)
        if with_bv:
            m["bv"] = bv[sl].reshape(1, DQ).astype(BF)
        in_maps.append(m)
    return in_maps, (with_bq, with_bk, with_bv)


def kernel(queries, keys, values, attn_mask, query_lengths, key_lengths,
           Wq, bq, Wk, bk, Wv, bv, Wo, bo):
    global LAST_RESULTS
    B = queries.shape[0]
    in_maps, bias_flags = _host_prep(queries, keys, values, key_lengths,
                                     Wq, bq, Wk, bk, Wv, bv, Wo)
    nc = _build_program(*bias_flags)
    res = run_bass_kernel_spmd(nc, in_maps, core_ids=list(range(2 * B)))
    LAST_RESULTS = res
    bo = np.asarray(bo, np.float32)
    out = np.zeros((B, T, D_MODEL), np.float32)
    for b in range(B):
        if int(key_lengths[b]) == 0:
            # kv/ksum are all-zero; reference output is exactly bo
            out[b] = bo[None, :]
        else:
            out[b] = (np.asarray(res.results[2 * b]["y"], np.float32)
                      + np.asarray(res.results[2 * b + 1]["y"], np.float32)
                      + bo)
    return out


# revision 14
# speedup vs baseline: 1.0679x; 1.0679x over previous
"""Trainium2 Bass kernel for MultiHeadLinearAttentionLayer (v3).

Problem (hardcoded shapes): B=4, L=S=2048, D_MODEL=1024, N_HEADS=16, HEAD_DIM=64.
  q/k/v = x @ W + b; RoPE(q), RoPE(k); qf/kf = elu(.)+1; kf masked by key_lengths;
  kv = kf^T v, ksum = sum kf; out = (qf @ kv) / (qf @ ksum + eps); y = out @ Wo + bo.

Sharding: 8 cores = 4 batches x 2 head-groups (8 heads each). Each core computes a
partial y (its head-group's contribution through Wo rows); host sums the two
partials per batch and adds bo. All matmuls bf16 (fp32 PSUM accumulation).

v3 design notes (HW-measured corrections over v2):
  - GpSimd compute is ELIMINATED: concurrent GpSimd tensor ops slow DVE ops
    ~6x (SBUF port contention, measured 334->2170ns), and GpSimd adds are
    ~2.2us/[128,1024] themselves.  GpSimd only issues table DMAs.
  - DVE broadcast APs with short runs are slow (16 sub-iterations ~2.5us), so
    the K cos table is a full [128,NT*DQ] table again; only the 32-wide sin
    half-swap tables keep broadcast form (their ops are short anyway).
  - elu(x)+1 = (x max 0) + exp(min(x,0)): min on DVE (4x tensor_scalar), exp
    on Scalar, and ONE DVE scalar_tensor_tensor for relu+add (no fast mode
    but a single pass).
  - Q RoPE: q2 = qt*cos + R@(qt*sin); rot half 0 is evacuated by Scalar so
    one q2 half-add runs in DVE 2x mode, the other reads PSUM directly.
  - Q/K process 1024-wide halfgroups (two PSUM banks per group; Q's
    k-outer/c-inner matmul order shares each LDWEIGHTS across 2 matmuls).
  - V-phase psv uses the separate "aux" PSUM tag so the K-phase tail and
    V-phase head don't serialize on the shared "mm" pool (which caused
    4-5us PE gaps + HAM re-throttle: matmuls measured at 427ns=cold clock).
  - V phase folds the key-length mask into the Scalar PSUM-evac via a
    per-partition scale AP.
  - attn tail: zes evac on Scalar; y stored bf16, one [128,1024] store per
    token tile.
"""

import os
import numpy as np
import ml_dtypes

import concourse.bacc as bacc
import concourse.mybir as mybir
from concourse import tile
from concourse.bass_utils import run_bass_kernel_spmd

BF16 = mybir.dt.bfloat16
F32 = mybir.dt.float32
AF = mybir.ActivationFunctionType
ALU = mybir.AluOpType
BF = ml_dtypes.bfloat16

D_MODEL = 1024
N_HEADS = 16
HEAD_DIM = 64
ROPE_THETA = 10000.0
T = 2048          # L = S
NT = T // 128     # 16 token tiles
NC_ = 4           # token chunks of 512
NK = D_MODEL // 128   # 8 contraction tiles
DQ = 512          # per-core head dims (8 heads x 64)
NJ = DQ // 128    # 4 dq tiles
NH = 8            # heads per core

LAST_RESULTS = None  # stashed BassKernelResults for test harnesses


def _build_program(with_bq, with_bk, with_bv):
    nc = bacc.Bacc("TRN2", target_bir_lowering=False)

    xqt_d = nc.declare_dram_parameter("xqt", [D_MODEL, T], BF16, isOutput=False)
    xkt_d = nc.declare_dram_parameter("xkt", [D_MODEL, T], BF16, isOutput=False)
    xvt_d = nc.declare_dram_parameter("xvt", [D_MODEL, T], BF16, isOutput=False)
    wq = nc.declare_dram_parameter("wq", [D_MODEL, DQ], BF16, isOutput=False)
    wk = nc.declare_dram_parameter("wk", [D_MODEL, DQ], BF16, isOutput=False)
    wv = nc.declare_dram_parameter("wv", [D_MODEL, DQ], BF16, isOutput=False)
    wo = nc.declare_dram_parameter("wo", [DQ, D_MODEL], BF16, isOutput=False)
    cosfm = nc.declare_dram_parameter("cosfm", [128, T], BF16, isOutput=False)
    sinfm = nc.declare_dram_parameter("sinfm", [128, T], BF16, isOutput=False)
    costm = nc.declare_dram_parameter("costm", [128, NT * DQ], BF16, isOutput=False)
    sintsm = nc.declare_dram_parameter("sintsm", [128, NT * 64], BF16, isOutput=False)
    rt = nc.declare_dram_parameter("rt", [128, 128], BF16, isOutput=False)
    eselp = nc.declare_dram_parameter("esel", [128, 128], BF16, isOutput=False)
    maskc = nc.declare_dram_parameter("maskc", [128, NT], F32, isOutput=False)
    bq = nc.declare_dram_parameter("bq", [1, DQ], BF16, isOutput=False) if with_bq else None
    bk = nc.declare_dram_parameter("bk", [1, DQ], BF16, isOutput=False) if with_bk else None
    bv = nc.declare_dram_parameter("bv", [1, DQ], BF16, isOutput=False) if with_bv else None
    y = nc.declare_dram_parameter("y", [T, D_MODEL], BF16, isOutput=True)

    with tile.TileContext(nc) as tc:
        with tc.tile_pool(name="sb", bufs=1) as sb, \
             tc.tile_pool(name="ps", bufs=1, space="PSUM") as ps:

            def tmp(name):
                return sb.tile([128, 1024], BF16, tag="tmp", bufs=11, name=name)

            # ---- constants / weights / Q inputs -------------------------
            # Split the critical startup DMAs across sync+vector queues
            # (each dma_start costs ~0.6us of issuing-queue time).
            wq_sb = sb.tile([128, NK, DQ], BF16, tag="w", bufs=3)
            xq = [sb.tile([128, T], BF16, tag="xt", bufs=16, name=f"xq{k}")
                  for k in range(NK)]
            for k in range(NK // 2):
                nc.sync.dma_start(wq_sb[:, k, :], wq[128 * k:128 * (k + 1), :])
                nc.sync.dma_start(xq[k][:], xqt_d[128 * k:128 * (k + 1), :])
            for k in range(NK // 2, NK):
                nc.gpsimd.dma_start(wq_sb[:, k, :], wq[128 * k:128 * (k + 1), :])
                nc.gpsimd.dma_start(xq[k][:], xqt_d[128 * k:128 * (k + 1), :])
            sinf = sb.tile([128, T], BF16, tag="fm", bufs=2, name="sinf")
            nc.scalar.dma_start(sinf[:], sinfm[:])
            cosf = sb.tile([128, T], BF16, tag="fm", bufs=2, name="cosf")
            nc.scalar.dma_start(cosf[:], cosfm[:])
            rt_sb = sb.tile([128, 128], BF16, tag="rt")
            nc.scalar.dma_start(rt_sb[:], rt[:])
            ones = sb.tile([1, 512], BF16, tag="ones")
            nc.vector.memset(ones[:], 1.0)
            zrow = sb.tile([1, 512], BF16, tag="zrow")
            nc.vector.memset(zrow[:], 0.0)
            if with_bq:
                bq_sb = sb.tile([1, DQ], BF16, tag="brow", bufs=3)
                nc.sync.dma_start(bq_sb[:], bq[:])

            qf_all = sb.tile([128, NJ, T], BF16, tag="qf")

            # ---- Q phase: 8 halfgroups (cp, j), 1024 tokens each --------
            NQG = 2 * NJ
            qst = [dict() for _ in range(NQG)]

            def q_A(i):      # 16 proj matmuls, k-outer/c-inner (LDW shared x2)
                cp, j = divmod(i, NJ)
                ps0 = ps.tile([128, 512], F32, tag="mm", bufs=5, name="psq0")
                ps1 = ps.tile([128, 512], F32, tag="mm", bufs=5, name="psq1")
                pss = (ps0, ps1)
                first = (True, True)
                if with_bq:
                    for ci in range(2):
                        nc.tensor.matmul(pss[ci][:],
                                         bq_sb[:, 128 * j:128 * (j + 1)],
                                         ones[:], start=True, stop=False)
                    first = (False, False)
                for k in range(NK):
                    for ci in range(2):
                        c = 2 * cp + ci
                        nc.tensor.matmul(
                            pss[ci][:], wq_sb[:, k, 128 * j:128 * (j + 1)],
                            xq[k][:, 512 * c:512 * (c + 1)],
                            start=(k == 0 and first[ci]), stop=(k == NK - 1))
                qst[i]["pss"] = pss

            def q_B(i):      # scalar evac of both banks -> qt [128,1024]
                ps0, ps1 = qst[i]["pss"]
                qt = tmp("qt")
                nc.scalar.copy(qt[:, 0:512], ps0[:])
                nc.scalar.copy(qt[:, 512:1024], ps1[:])
                qst[i]["qt"] = qt

            def q_C(i):      # ts = qt*sin (V 2x); rot = R@ts (PE); t1 = qt*cos
                cp, j = divmod(i, NJ)
                ch = slice(1024 * cp, 1024 * (cp + 1))
                qt = qst[i]["qt"]
                ts = tmp("ts")
                nc.vector.tensor_tensor(ts[:], qt[:], sinf[:, ch], ALU.mult)
                rot0 = ps.tile([128, 512], F32, tag="aux", bufs=2, name="rot0")
                rot1 = ps.tile([128, 512], F32, tag="aux", bufs=2, name="rot1")
                nc.tensor.matmul(rot0[:], rt_sb[:], ts[:, 0:512],
                                 start=True, stop=True)
                nc.tensor.matmul(rot1[:], rt_sb[:], ts[:, 512:1024],
                                 start=True, stop=True)
                t1 = tmp("t1")
                nc.vector.tensor_tensor(t1[:], qt[:], cosf[:, ch], ALU.mult)
                qst[i]["rot"] = (rot0, rot1)
                qst[i]["t1"] = t1

            def q_D(i):      # scalar evac of rot half 0 -> rs0 (bf16 SBUF)
                rot0, _ = qst[i]["rot"]
                rs0 = tmp("rs0")
                nc.scalar.copy(rs0[:, 0:512], rot0[:])
                qst[i]["rs0"] = rs0

            def q_E(i):      # q2 halves; q2m = min(q2,0) (V 4x)
                _, rot1 = qst[i]["rot"]
                t1 = qst[i]["t1"]
                rs0 = qst[i]["rs0"]
                q2 = tmp("q2")
                nc.vector.tensor_tensor(q2[:, 512:1024], t1[:, 512:1024],
                                        rot1[:], ALU.add)
                nc.vector.tensor_tensor(q2[:, 0:512], t1[:, 0:512],
                                        rs0[:, 0:512], ALU.add)
                q2m = tmp("q2m")
                nc.vector.tensor_scalar_min(q2m[:], q2[:], 0.0)
                qst[i]["q2"] = q2
                qst[i]["q2m"] = q2m

            def q_F(i):      # qe = exp(q2m) (S)
                qe = tmp("qe")
                nc.scalar.activation(qe[:], qst[i]["q2m"][:], AF.Exp)
                qst[i]["qe"] = qe

            def q_G(i):      # qf = (q2 max 0) + qe  (one V stt pass)
                cp, j = divmod(i, NJ)
                ch = slice(1024 * cp, 1024 * (cp + 1))
                nc.vector.scalar_tensor_tensor(
                    qf_all[:, j, ch], qst[i]["q2"][:], 0.0, qst[i]["qe"][:],
                    ALU.max, ALU.add)
                qst[i].clear()

            with nc.named_scope("qproj"):
                for s in range(NQG + 5):
                    if s < NQG:
                        q_A(s)
                    if 0 <= s - 1 < NQG:
                        q_B(s - 1)
                    if 0 <= s - 2 < NQG:
                        q_D(s - 2)
                    if 0 <= s - 4 < NQG:
                        q_G(s - 4)
                    if 0 <= s - 2 < NQG:
                        q_E(s - 2)
                    if 0 <= s - 3 < NQG:
                        q_F(s - 3)
                    if 0 <= s - 1 < NQG:
                        q_C(s - 1)

            # ---- K phase: 8 halfgroups of 2 token tiles -----------------
            wk_sb = sb.tile([128, NK, DQ], BF16, tag="w", bufs=3)
            xk = [sb.tile([128, T], BF16, tag="xt", bufs=16, name=f"xk{k}")
                  for k in range(NK)]
            for k in range(NK):
                nc.sync.dma_start(wk_sb[:, k, :], wk[128 * k:128 * (k + 1), :])
                nc.sync.dma_start(xk[k][:], xkt_d[128 * k:128 * (k + 1), :])
            cost_sb = sb.tile([128, NT, DQ], BF16, tag="tcos", bufs=1)
            nc.gpsimd.dma_start(cost_sb[:],
                                costm.rearrange("p (m c) -> p m c", m=NT))
            sint_sb = sb.tile([128, NT, 64], BF16, tag="tsin", bufs=1)
            nc.gpsimd.dma_start(sint_sb[:],
                                sintsm.rearrange("p (m c) -> p m c", m=NT))
            if with_bk:
                bk_sb = sb.tile([1, DQ], BF16, tag="brow", bufs=3)
                nc.sync.dma_start(bk_sb[:], bk[:])

            kfp = [sb.tile([128, 1024], BF16, tag="kf", bufs=8, name=f"kfp{g}")
                   for g in range(NT // 2)]
            NKG = NT // 2
            kst = [dict() for _ in range(NKG)]

            def k_A(g):      # 16 proj matmuls for token tiles 2g, 2g+1
                ps0 = ps.tile([128, 512], F32, tag="mm", bufs=5, name="psk0")
                ps1 = ps.tile([128, 512], F32, tag="mm", bufs=5, name="psk1")
                pss = (ps0, ps1)
                first = [True, True]
                if with_bk:
                    for mi in range(2):
                        nc.tensor.matmul(pss[mi][:], ones[:, 0:128], bk_sb[:],
                                         start=True, stop=False)
                        first[mi] = False
                for k in range(NK):
                    for mi in range(2):
                        m = 2 * g + mi
                        nc.tensor.matmul(
                            pss[mi][:], xk[k][:, 128 * m:128 * (m + 1)],
                            wk_sb[:, k, :],
                            start=(k == 0 and first[mi]), stop=(k == NK - 1))
                kst[g]["pss"] = pss

            def k_B(g):      # scalar evac -> kp [128, 2x512]
                ps0, ps1 = kst[g]["pss"]
                kp = tmp("kp")
                nc.scalar.copy(kp[:, 0:512], ps0[:])
                nc.scalar.copy(kp[:, 512:1024], ps1[:])
                kst[g]["kp"] = kp

            def k_C(g):      # t1 = kp*cos (V 2x, full table); t2 half-swaps
                kp = kst[g]["kp"]
                t1 = tmp("t1k")
                nc.vector.tensor_tensor(
                    t1.rearrange("p (m c) -> p m c", m=2)[:],
                    kp.rearrange("p (m c) -> p m c", m=2)[:],
                    cost_sb[:, 2 * g:2 * g + 2, :], ALU.mult)
                t2 = tmp("t2k")
                for mi in range(2):
                    m = 2 * g + mi
                    k8 = kp[:, 512 * mi:512 * (mi + 1)] \
                        .rearrange("p (h s i) -> p h s i", h=NH, s=2, i=32)
                    t8 = t2[:, 512 * mi:512 * (mi + 1)] \
                        .rearrange("p (h s i) -> p h s i", h=NH, s=2, i=32)
                    sa = sint_sb[:, m, 0:32] \
                        .rearrange("p (a i) -> p a i", a=1) \
                        .broadcast_to([128, NH, 32])
                    sb_ = sint_sb[:, m, 32:64] \
                        .rearrange("p (a i) -> p a i", a=1) \
                        .broadcast_to([128, NH, 32])
                    nc.vector.tensor_tensor(t8[:, :, 0, :], k8[:, :, 1, :],
                                            sa[:], ALU.mult)
                    nc.vector.tensor_tensor(t8[:, :, 1, :], k8[:, :, 0, :],
                                            sb_[:], ALU.mult)
                kst[g]["t1"] = t1
                kst[g]["t2"] = t2

            def k_D(g):      # k2 = t1 + t2 (V 2x); k2m = min(k2,0) (V 4x)
                k2 = tmp("k2")
                nc.vector.tensor_tensor(k2[:], kst[g]["t1"][:], kst[g]["t2"][:],
                                        ALU.add)
                k2m = tmp("k2m")
                nc.vector.tensor_scalar_min(k2m[:], k2[:], 0.0)
                kst[g]["k2"] = k2
                kst[g]["k2m"] = k2m

            def k_E(g):      # ke = exp(k2m) (S)
                ke = tmp("ke")
                nc.scalar.activation(ke[:], kst[g]["k2m"][:], AF.Exp)
                kst[g]["ke"] = ke

            def k_F(g):      # kf = (k2 max 0) + ke  (one V stt pass)
                nc.vector.scalar_tensor_tensor(
                    kfp[g][:], kst[g]["k2"][:], 0.0, kst[g]["ke"][:],
                    ALU.max, ALU.add)
                kst[g].clear()

            with nc.named_scope("kproj"):
                for s in range(NKG + 5):
                    if s < NKG:
                        k_A(s)
                    if 0 <= s - 1 < NKG:
                        k_B(s - 1)
                    if 0 <= s - 4 < NKG:
                        k_F(s - 4)
                    if 0 <= s - 2 < NKG:
                        k_D(s - 2)
                    if 0 <= s - 3 < NKG:
                        k_E(s - 3)
                    if 0 <= s - 1 < NKG:
                        k_C(s - 1)

            # ---- V phase + kv accumulation ------------------------------
            wv_sb = sb.tile([128, NK, DQ], BF16, tag="w", bufs=3)
            mk_sb = sb.tile([128, NT], F32, tag="mask")
            nc.gpsimd.dma_start(mk_sb[:], maskc[:])
            wo_sb = sb.tile([128, NJ, D_MODEL], BF16, tag="wo")
            esel = sb.tile([128, 128], BF16, tag="esel")
            xv = [sb.tile([128, T], BF16, tag="xt", bufs=16, name=f"xv{k}")
                  for k in range(NK)]
            nc.gpsimd.dma_start(wv_sb[:], wv.rearrange("(k p) c -> p k c", p=128))
            for k in range(NK):
                nc.sync.dma_start(xv[k][:], xvt_d[128 * k:128 * (k + 1), :])
            nc.gpsimd.dma_start(wo_sb[:], wo.rearrange("(k p) c -> p k c", p=128))
            nc.gpsimd.dma_start(esel[:], eselp[:])
            if with_bv:
                bv_sb = sb.tile([1, DQ], BF16, tag="brow", bufs=3)
                nc.sync.dma_start(bv_sb[:], bv[:])

            kvp_t = ps.tile([128, 512], F32, tag="kv", bufs=1, name="kvp")
            kvp = kvp_t[:, 0:272]
            # open the kv accumulation group: zero the whole region so later
            # disjoint-region matmuls (start=False) all accumulate onto it
            nc.tensor.matmul(kvp[:], zrow[:, 0:128], zrow[:, 0:272],
                             start=True, stop=False)
            vst = [dict() for _ in range(NT)]

            def v_A(m):
                psv = ps.tile([128, 512], F32, tag="aux", bufs=2, name="psv")
                first = True
                if with_bv:
                    nc.tensor.matmul(psv[:], ones[:, 0:128], bv_sb[:],
                                     start=True, stop=False)
                    first = False
                for k in range(NK):
                    nc.tensor.matmul(
                        psv[:], xv[k][:, 128 * m:128 * (m + 1)],
                        wv_sb[:, k, :], start=first, stop=(k == NK - 1))
                    first = False
                vst[m]["psv"] = psv

            def v_B(m):      # scalar evac with fused mask scale; V mask col
                psv = vst[m]["psv"]
                v2 = sb.tile([128, NH, 68], BF16, tag="vv", bufs=4, name="v2")
                nc.scalar.activation(
                    v2[:, :, 0:64], psv.rearrange("p (h i) -> p h i", h=NH),
                    AF.Copy, scale=mk_sb[:, m:m + 1])
                nc.vector.tensor_copy(
                    v2[:, :, 64:65],
                    mk_sb[:, m:m + 1].rearrange("p (a i) -> p a i", a=1)
                    .broadcast_to([128, NH, 1]))
                vst[m]["v2"] = v2

            def v_C(m):      # kv' accumulation
                v2 = vst[m]["v2"]
                for h in range(NH):
                    r0 = 64 * (h % 2)
                    c0 = 68 * (h // 2)
                    nc.tensor.matmul(
                        kvp[r0:r0 + 64, c0:c0 + 68],
                        kfp[m // 2][:, 512 * (m % 2) + 64 * h:
                                     512 * (m % 2) + 64 * (h + 1)],
                        v2[:, h, :],
                        start=False, stop=False,
                        tile_position=(0, r0))
                vst[m].clear()

            with nc.named_scope("vproj"):
                for s in range(NT + 1):
                    if s < NT:
                        v_A(s)
                    if 0 <= s - 1 < NT:
                        v_B(s - 1)
                        v_C(s - 1)
            # close the kv group (single dep covering all kv matmuls)
            nc.tensor.matmul(kvp[:], zrow[:, 0:128], zrow[:, 0:272],
                             start=False, stop=True)

            # repack kv' into per-j block-diagonal [128,128] + ksum pack [128,8]
            kvblk = [sb.tile([128, 128], BF16, tag="kvb", bufs=NJ, name=f"kvb{j}")
                     for j in range(NJ)]
            kspack = sb.tile([128, 8], BF16, tag="ksp")
            nc.vector.memset(kspack[:], 0.0)
            for j in range(NJ):
                nc.vector.memset(kvblk[j][:], 0.0)
                nc.vector.tensor_copy(kvblk[j][0:64, 0:64],
                                      kvp[0:64, 68 * j:68 * j + 64])
                nc.vector.tensor_copy(kvblk[j][64:128, 64:128],
                                      kvp[64:128, 68 * j:68 * j + 64])
                nc.vector.tensor_copy(kspack[0:64, 2 * j:2 * j + 1],
                                      kvp[0:64, 68 * j + 64:68 * j + 65])
                nc.vector.tensor_copy(kspack[64:128, 2 * j + 1:2 * j + 2],
                                      kvp[64:128, 68 * j + 64:68 * j + 65])

            # ---- attention (feature-major) + output projection ----------
            ast = [dict() for _ in range(NC_)]

            def a_den(c):
                ch = slice(512 * c, 512 * (c + 1))
                den = ps.tile([128, 512], F32, tag="aux", bufs=2, name="den")
                for j in range(NJ):
                    nc.tensor.matmul(den[32 * j:32 * j + 2, :],
                                     kspack[:, 2 * j:2 * j + 2],
                                     qf_all[:, j, ch], start=True, stop=True,
                                     tile_position=(0, 32 * j))
                zrecf = sb.tile([128, 512], F32, tag="zrf", bufs=2, name="zrecf")
                with nc.allow_low_precision(reason="z scale approx is fine"):
                    nc.vector.reciprocal_approx_fast(zrecf[:], den[:])
                zrec = sb.tile([128, 512], BF16, tag="zr", bufs=2, name="zrec")
                nc.vector.tensor_copy(zrec[:], zrecf[:])
                ast[c]["zrec"] = zrec

            def a_attn(c):
                ch = slice(512 * c, 512 * (c + 1))
                zrec = ast[c]["zrec"]
                osbs = []
                for j in range(NJ):
                    zep = ps.tile([128, 512], F32, tag="mm", bufs=5, name="zep")
                    nc.tensor.matmul(zep[:], esel[32 * j:32 * j + 2, :],
                                     zrec[32 * j:32 * j + 2, :],
                                     start=True, stop=True,
                                     tile_position=(32 * j, 0))
                    zes = sb.tile([128, 512], BF16, tag="ze", bufs=4,
                                  name="zes")
                    nc.scalar.copy(zes[:], zep[:])
                    opp = ps.tile([128, 512], F32, tag="mm", bufs=5, name="opp")
                    nc.tensor.matmul(opp[:], kvblk[j][:], qf_all[:, j, ch],
                                     start=True, stop=True)
                    osb = sb.tile([128, 512], BF16, tag="osb", bufs=8,
                                  name=f"osb{j}")
                    nc.vector.tensor_tensor(osb[:], opp[:], zes[:], ALU.mult)
                    osbs.append(osb)
                ast[c]["osbs"] = osbs

            def a_y(c):
                osbs = ast[c]["osbs"]
                for mm_ in range(4):
                    m = 4 * c + mm_
                    yps = [ps.tile([128, 512], F32, tag="mm", bufs=5, name="yp")
                           for _ in range(2)]
                    for j in range(NJ):
                        for c2 in range(2):
                            nc.tensor.matmul(
                                yps[c2][:],
                                osbs[j][:, 128 * mm_:128 * (mm_ + 1)],
                                wo_sb[:, j, 512 * c2:512 * (c2 + 1)],
                                start=(j == 0), stop=(j == NJ - 1))
                    ysb = sb.tile([128, 1024], BF16, tag="ysb", bufs=3,
                                  name="ysb")
                    nc.scalar.copy(ysb[:, 0:512], yps[0][:])
                    if mm_ == 3:
                        nc.vector.tensor_copy(ysb[:, 512:1024], yps[1][:])
                    else:
                        nc.scalar.copy(ysb[:, 512:1024], yps[1][:])
                    nc.sync.dma_start(y[128 * m:128 * (m + 1), :], ysb[:])
                ast[c].clear()

            with nc.named_scope("attn"):
                for s in range(NC_ + 1):
                    if s < NC_:
                        a_den(s)
                    if 0 <= s - 1 < NC_:
                        a_attn(s - 1)
                        a_y(s - 1)

    nc.compile()
    return nc


def _host_prep(queries, keys, values, key_lengths, Wq, bq, Wk, bk, Wv, bv, Wo):
    """Build the per-core input maps (host side: transpose, cast, tables)."""
    B = queries.shape[0]
    # per-head [evens|odds] feature permutation
    pat = np.concatenate([np.arange(0, HEAD_DIM, 2), np.arange(1, HEAD_DIM, 2)])
    perm = np.concatenate([h * HEAD_DIM + pat for h in range(NH)])  # within 512

    inv_freq = 1.0 / (ROPE_THETA ** (np.arange(0, HEAD_DIM, 2, dtype=np.float64)
                                     / HEAD_DIM))  # [32]
    t = np.arange(T, dtype=np.float64)
    ang = t[:, None] * inv_freq[None, :]           # [T, 32]
    cos32 = np.cos(ang).astype(np.float32)
    sin32 = np.sin(ang).astype(np.float32)

    # feature-major tables [128, T]: row r: block = r % 64; i = block % 32
    idx = np.arange(128) % HEAD_DIM
    fidx = np.where(idx < 32, idx, idx - 32)
    cosfm = cos32[:, fidx].T.astype(BF)            # [128, T]
    sinfm = sin32[:, fidx].T.astype(BF)

    # token-major full cos table [128, NT*DQ] (per m: [128 tok, 512 feat]);
    # small sin table [128, NT*64]: [0:32]=-sin, [32:64]=+sin per token
    cidx = np.arange(DQ) % HEAD_DIM
    cf = np.where(cidx < 32, cidx, cidx - 32)
    costm_full = cos32[:, cf]                      # [T, 512]
    costm = np.ascontiguousarray(
        costm_full.reshape(NT, 128, DQ).transpose(1, 0, 2)
        .reshape(128, NT * DQ)).astype(BF)
    sin_t = sin32.reshape(NT, 128, 32).transpose(1, 0, 2)    # [128, NT, 32]
    sintsm = np.concatenate([-sin_t, sin_t], axis=2)         # [128, NT, 64]
    sintsm = np.ascontiguousarray(sintsm.reshape(128, NT * 64)).astype(BF)

    # rotation matrix R (permuted layout), lhsT = R.T
    R = np.zeros((128, 128), np.float32)
    for base in (0, 64):
        R[base + 0:base + 32, base + 32:base + 64] = -np.eye(32)
        R[base + 32:base + 64, base + 0:base + 32] = np.eye(32)
    rt = np.ascontiguousarray(R.T).astype(BF)
    esel = np.zeros((128, 128), np.float32)
    for j in range(NJ):
        esel[32 * j, 0:64] = 1.0
        esel[32 * j + 1, 64:128] = 1.0
    esel = esel.astype(BF)

    with_bq = bool(np.any(np.asarray(bq)))
    with_bk = bool(np.any(np.asarray(bk)))
    with_bv = bool(np.any(np.asarray(bv)))

    xqt_b = [np.ascontiguousarray(np.asarray(queries[b]).astype(BF).T)
             for b in range(B)]
    xkt_b = [np.ascontiguousarray(np.asarray(keys[b]).astype(BF).T)
             for b in range(B)]
    xvt_b = [np.ascontiguousarray(np.asarray(values[b]).astype(BF).T)
             for b in range(B)]
    Wq = np.asarray(Wq, np.float32); Wk = np.asarray(Wk, np.float32)
    Wv = np.asarray(Wv, np.float32); Wo = np.asarray(Wo, np.float32)
    bq = np.asarray(bq, np.float32); bk = np.asarray(bk, np.float32)
    bv = np.asarray(bv, np.float32)

    in_maps = []
    for core in range(2 * B):
        b, g = core // 2, core % 2
        sl = slice(DQ * g, DQ * (g + 1))
        mask = (np.arange(T) < int(key_lengths[b])).astype(np.float32)
        maskc = np.ascontiguousarray(mask.reshape(NT, 128).T)
        m = {
            "xqt": xqt_b[b], "xkt": xkt_b[b], "xvt": xvt_b[b],
            "wq": np.ascontiguousarray(Wq[:, sl][:, perm]).astype(BF),
            "wk": np.ascontiguousarray(Wk[:, sl][:, perm]).astype(BF),
            "wv": np.ascontiguousarray(Wv[:, sl]).astype(BF),
            "wo": np.ascontiguousarray(Wo[sl, :]).astype(BF),
            "cosfm": cosfm, "sinfm": sinfm,
            "costm": costm, "sintsm": sintsm,
            "rt": rt, "esel": esel, "maskc": maskc,
        }
        if with_bq:
            m["bq"] = bq[sl][perm].reshape(1, DQ).astype(BF)
        if with_bk:
            m["bk"] = bk[sl][perm].reshape(1, DQ).astype(BF)
        if with_bv:
            m["bv"] = bv[sl].reshape(1, DQ).astype(BF)
        in_maps.append(m)
    return in_maps, (with_bq, with_bk, with_bv)


def kernel(queries, keys, values, attn_mask, query_lengths, key_lengths,
           Wq, bq, Wk, bk, Wv, bv, Wo, bo):
    global LAST_RESULTS
    B = queries.shape[0]
    in_maps, bias_flags = _host_prep(queries, keys, values, key_lengths,
                                     Wq, bq, Wk, bk, Wv, bv, Wo)
    nc = _build_program(*bias_flags)
    res = run_bass_kernel_spmd(nc, in_maps, core_ids=list(range(2 * B)))
    LAST_RESULTS = res
    bo = np.asarray(bo, np.float32)
    out = np.zeros((B, T, D_MODEL), np.float32)
    for b in range(B):
        if int(key_lengths[b]) == 0:
            # kv/ksum are all-zero; reference output is exactly bo
            out[b] = bo[None, :]
        else:
            out[b] = (np.asarray(res.results[2 * b]["y"], np.float32)
                      + np.asarray(res.results[2 * b + 1]["y"], np.float32)
                      + bo)
    return out


# revision 16
# speedup vs baseline: 1.1753x; 1.1006x over previous
"""Trainium2 Bass kernel for MultiHeadLinearAttentionLayer (v3).

Problem (hardcoded shapes): B=4, L=S=2048, D_MODEL=1024, N_HEADS=16, HEAD_DIM=64.
  q/k/v = x @ W + b; RoPE(q), RoPE(k); qf/kf = elu(.)+1; kf masked by key_lengths;
  kv = kf^T v, ksum = sum kf; out = (qf @ kv) / (qf @ ksum + eps); y = out @ Wo + bo.

Sharding: 8 cores = 4 batches x 2 head-groups (8 heads each). Each core computes a
partial y (its head-group's contribution through Wo rows); host sums the two
partials per batch and adds bo. All matmuls bf16 (fp32 PSUM accumulation).

v3 design notes (HW-measured corrections over v2):
  - GpSimd compute is ELIMINATED: concurrent GpSimd tensor ops slow DVE ops
    ~6x (SBUF port contention, measured 334->2170ns), and GpSimd adds are
    ~2.2us/[128,1024] themselves.  GpSimd only issues table DMAs.
  - DVE broadcast APs with short runs are slow (16 sub-iterations ~2.5us), so
    the K cos table is a full [128,NT*DQ] table again; only the 32-wide sin
    half-swap tables keep broadcast form (their ops are short anyway).
  - elu(x)+1 = (x max 0) + exp(min(x,0)): min on DVE (4x tensor_scalar), exp
    on Scalar, and ONE DVE scalar_tensor_tensor for relu+add (no fast mode
    but a single pass).
  - Q RoPE: q2 = qt*cos + R@(qt*sin); rot half 0 is evacuated by Scalar so
    one q2 half-add runs in DVE 2x mode, the other reads PSUM directly.
  - Q/K process 1024-wide halfgroups (two PSUM banks per group; Q's
    k-outer/c-inner matmul order shares each LDWEIGHTS across 2 matmuls).
  - V-phase psv uses the separate "aux" PSUM tag so the K-phase tail and
    V-phase head don't serialize on the shared "mm" pool (which caused
    4-5us PE gaps + HAM re-throttle: matmuls measured at 427ns=cold clock).
  - V phase folds the key-length mask into the Scalar PSUM-evac via a
    per-partition scale AP.
  - attn tail: zes evac on Scalar; y stored bf16, one [128,1024] store per
    token tile.
"""

import os
import numpy as np
import ml_dtypes

import concourse.bacc as bacc
import concourse.mybir as mybir
from concourse import tile
from concourse.bass_utils import run_bass_kernel_spmd

BF16 = mybir.dt.bfloat16
FP8 = mybir.dt.float8e4
F32 = mybir.dt.float32
DR = mybir.MatmulPerfMode.DoubleRow
AF = mybir.ActivationFunctionType
ALU = mybir.AluOpType
BF = ml_dtypes.bfloat16

D_MODEL = 1024
N_HEADS = 16
HEAD_DIM = 64
ROPE_THETA = 10000.0
T = 2048          # L = S
NT = T // 128     # 16 token tiles
NC_ = 4           # token chunks of 512
NK = D_MODEL // 128   # 8 contraction tiles
DQ = 512          # per-core head dims (8 heads x 64)
NJ = DQ // 128    # 4 dq tiles
NH = 8            # heads per core

LAST_RESULTS = None  # stashed BassKernelResults for test harnesses


def _build_program(with_bq, with_bk, with_bv, qdescale, kdescale):
    nc = bacc.Bacc("TRN2", target_bir_lowering=False)

    xqt_d = nc.declare_dram_parameter("xqt", [D_MODEL, T], FP8, isOutput=False)
    xkt_d = nc.declare_dram_parameter("xkt", [D_MODEL, T], FP8, isOutput=False)
    xvt_d = nc.declare_dram_parameter("xvt", [D_MODEL, T], BF16, isOutput=False)
    wq = nc.declare_dram_parameter("wq", [D_MODEL, DQ], FP8, isOutput=False)
    wk = nc.declare_dram_parameter("wk", [D_MODEL, DQ], FP8, isOutput=False)
    wv = nc.declare_dram_parameter("wv", [D_MODEL, DQ], BF16, isOutput=False)
    wo = nc.declare_dram_parameter("wo", [DQ, D_MODEL], BF16, isOutput=False)
    cosfm = nc.declare_dram_parameter("cosfm", [128, T], BF16, isOutput=False)
    sinfm = nc.declare_dram_parameter("sinfm", [128, T], BF16, isOutput=False)
    costm = nc.declare_dram_parameter("costm", [128, NT * DQ], BF16, isOutput=False)
    sintsm = nc.declare_dram_parameter("sintsm", [128, NT * 64], BF16, isOutput=False)
    rt = nc.declare_dram_parameter("rt", [128, 128], BF16, isOutput=False)
    eselp = nc.declare_dram_parameter("esel", [128, 128], BF16, isOutput=False)
    maskc = nc.declare_dram_parameter("maskc", [128, NT], F32, isOutput=False)
    bq = nc.declare_dram_parameter("bq", [1, DQ], BF16, isOutput=False) if with_bq else None
    bk = nc.declare_dram_parameter("bk", [1, DQ], BF16, isOutput=False) if with_bk else None
    bv = nc.declare_dram_parameter("bv", [1, DQ], BF16, isOutput=False) if with_bv else None
    y = nc.declare_dram_parameter("y", [T, D_MODEL], BF16, isOutput=True)

    with tile.TileContext(nc) as tc:
        with tc.tile_pool(name="sb", bufs=1) as sb, \
             tc.tile_pool(name="ps", bufs=1, space="PSUM") as ps:

            def tmp(name):
                return sb.tile([128, 1024], BF16, tag="tmp", bufs=14, name=name)

            # ---- constants / weights / Q inputs -------------------------
            # Split the critical startup DMAs across sync+vector queues
            # (each dma_start costs ~0.6us of issuing-queue time).
            wq_sb = sb.tile([128, NK, DQ], FP8, tag="w8", bufs=2)
            xq_t = sb.tile([128, NK, T], FP8, tag="xt8", bufs=2, name="xq")
            for k in range(NK):
                nc.sync.dma_start(wq_sb[:, k, :], wq[128 * k:128 * (k + 1), :])
            for k in range(NK // 2):
                nc.sync.dma_start(xq_t[:, k, :], xqt_d[128 * k:128 * (k + 1), :])
            sinf = sb.tile([128, T], BF16, tag="fm", bufs=2, name="sinf")
            nc.scalar.dma_start(sinf[:], sinfm[:])
            for k in range(NK // 2, NK):
                nc.scalar.dma_start(xq_t[:, k, :],
                                    xqt_d[128 * k:128 * (k + 1), :])
            cosf = sb.tile([128, T], BF16, tag="fm", bufs=2, name="cosf")
            nc.scalar.dma_start(cosf[:], cosfm[:])
            rt_sb = sb.tile([128, 128], BF16, tag="rt")
            nc.scalar.dma_start(rt_sb[:], rt[:])
            ones = sb.tile([1, 512], BF16, tag="ones")
            nc.vector.memset(ones[:], 1.0)
            zrow = sb.tile([1, 512], BF16, tag="zrow")
            nc.vector.memset(zrow[:], 0.0)
            if with_bq:
                bq_sb = sb.tile([1, DQ], BF16, tag="brow", bufs=3)
                nc.sync.dma_start(bq_sb[:], bq[:])

            qf_all = sb.tile([128, NJ, T], BF16, tag="qf")

            # ---- Q phase: 8 halfgroups (cp, j), 1024 tokens each --------
            NQG = 2 * NJ
            qst = [dict() for _ in range(NQG)]

            def q_A(i):      # 16 proj matmuls, k-outer/c-inner (LDW shared x2)
                cp, j = divmod(i, NJ)
                ps0 = ps.tile([128, 512], F32, tag="mm", bufs=5, name="psq0")
                ps1 = ps.tile([128, 512], F32, tag="mm", bufs=5, name="psq1")
                pss = (ps0, ps1)
                first = (True, True)
                if with_bq:
                    for ci in range(2):
                        nc.tensor.matmul(pss[ci][:],
                                         bq_sb[:, 128 * j:128 * (j + 1)],
                                         ones[:], start=True, stop=False)
                    first = (False, False)
                for kp in range(NK // 2):
                    for ci in range(2):
                        c = 2 * cp + ci
                        nc.tensor.matmul(
                            pss[ci][:],
                            wq_sb[:, 2 * kp:2 * kp + 2, 128 * j:128 * (j + 1)],
                            xq_t[:, 2 * kp:2 * kp + 2, 512 * c:512 * (c + 1)],
                            start=(kp == 0 and first[ci]),
                            stop=(kp == NK // 2 - 1), perf_mode=DR)
                qst[i]["pss"] = pss

            def q_B(i):      # scalar evac of both banks -> qt [128,1024]
                ps0, ps1 = qst[i]["pss"]
                qt = tmp("qt")
                nc.scalar.activation(qt[:, 0:512], ps0[:], AF.Copy,
                                     scale=qdescale)
                nc.scalar.activation(qt[:, 512:1024], ps1[:], AF.Copy,
                                     scale=qdescale)
                qst[i]["qt"] = qt

            def q_C(i):      # ts = qt*sin (V 2x); rot = R@ts (PE); t1 = qt*cos
                cp, j = divmod(i, NJ)
                ch = slice(1024 * cp, 1024 * (cp + 1))
                qt = qst[i]["qt"]
                ts = tmp("ts")
                nc.vector.tensor_tensor(ts[:], qt[:], sinf[:, ch], ALU.mult)
                rot0 = ps.tile([128, 512], F32, tag="aux", bufs=2, name="rot0")
                rot1 = ps.tile([128, 512], F32, tag="aux", bufs=2, name="rot1")
                nc.tensor.matmul(rot0[:], rt_sb[:], ts[:, 0:512],
                                 start=True, stop=True)
                nc.tensor.matmul(rot1[:], rt_sb[:], ts[:, 512:1024],
                                 start=True, stop=True)
                t1 = tmp("t1")
                nc.vector.tensor_tensor(t1[:], qt[:], cosf[:, ch], ALU.mult)
                qst[i]["rot"] = (rot0, rot1)
                qst[i]["t1"] = t1

            def q_D(i):      # scalar evac of rot half 0 -> rs0 (bf16 SBUF)
                rot0, _ = qst[i]["rot"]
                rs0 = tmp("rs0")
                nc.scalar.copy(rs0[:, 0:512], rot0[:])
                qst[i]["rs0"] = rs0

            def q_E(i):      # q2 halves; q2m = min(q2,0) (V 4x)
                _, rot1 = qst[i]["rot"]
                t1 = qst[i]["t1"]
                rs0 = qst[i]["rs0"]
                q2 = tmp("q2")
                nc.vector.tensor_tensor(q2[:, 512:1024], t1[:, 512:1024],
                                        rot1[:], ALU.add)
                nc.vector.tensor_tensor(q2[:, 0:512], t1[:, 0:512],
                                        rs0[:, 0:512], ALU.add)
                q2m = tmp("q2m")
                nc.vector.tensor_scalar_min(q2m[:], q2[:], 0.0)
                qst[i]["q2"] = q2
                qst[i]["q2m"] = q2m

            def q_F(i):      # qe = exp(q2m) (S)
                qe = tmp("qe")
                nc.scalar.activation(qe[:], qst[i]["q2m"][:], AF.Exp)
                qst[i]["qe"] = qe

            def q_G(i):      # qf = (q2 max 0) + qe  (one V stt pass)
                cp, j = divmod(i, NJ)
                ch = slice(1024 * cp, 1024 * (cp + 1))
                nc.vector.scalar_tensor_tensor(
                    qf_all[:, j, ch], qst[i]["q2"][:], 0.0, qst[i]["qe"][:],
                    ALU.max, ALU.add)
                qst[i].clear()

            with nc.named_scope("qproj"):
                for s in range(NQG + 5):
                    if s < NQG:
                        q_A(s)
                    if 0 <= s - 1 < NQG:
                        q_B(s - 1)
                    if 0 <= s - 2 < NQG:
                        q_D(s - 2)
                    if 0 <= s - 4 < NQG:
                        q_G(s - 4)
                    if 0 <= s - 2 < NQG:
                        q_E(s - 2)
                    if 0 <= s - 3 < NQG:
                        q_F(s - 3)
                    if 0 <= s - 1 < NQG:
                        q_C(s - 1)

            # ---- K phase: 8 halfgroups of 2 token tiles -----------------
            wk_sb = sb.tile([128, NK, DQ], FP8, tag="w8", bufs=2)
            xk_t = sb.tile([128, NK, T], FP8, tag="xt8", bufs=2, name="xk")
            for k in range(NK):
                nc.sync.dma_start(wk_sb[:, k, :], wk[128 * k:128 * (k + 1), :])
                nc.sync.dma_start(xk_t[:, k, :], xkt_d[128 * k:128 * (k + 1), :])
            cost_sb = sb.tile([128, NT, DQ], BF16, tag="tcos", bufs=1)
            nc.gpsimd.dma_start(cost_sb[:],
                                costm.rearrange("p (m c) -> p m c", m=NT))
            sint_sb = sb.tile([128, NT, 64], BF16, tag="tsin", bufs=1)
            nc.gpsimd.dma_start(sint_sb[:],
                                sintsm.rearrange("p (m c) -> p m c", m=NT))
            if with_bk:
                bk_sb = sb.tile([1, DQ], BF16, tag="brow", bufs=3)
                nc.sync.dma_start(bk_sb[:], bk[:])

            kfp = [sb.tile([128, 1024], BF16, tag="kf", bufs=8, name=f"kfp{g}")
                   for g in range(NT // 2)]
            NKG = NT // 2
            kst = [dict() for _ in range(NKG)]

            def k_A(g):      # 16 proj matmuls for token tiles 2g, 2g+1
                ps0 = ps.tile([128, 512], F32, tag="mm", bufs=5, name="psk0")
                ps1 = ps.tile([128, 512], F32, tag="mm", bufs=5, name="psk1")
                pss = (ps0, ps1)
                first = [True, True]
                if with_bk:
                    for mi in range(2):
                        nc.tensor.matmul(pss[mi][:], ones[:, 0:128], bk_sb[:],
                                         start=True, stop=False)
                        first[mi] = False
                for kp in range(NK // 2):
                    for mi in range(2):
                        m = 2 * g + mi
                        nc.tensor.matmul(
                            pss[mi][:],
                            xk_t[:, 2 * kp:2 * kp + 2, 128 * m:128 * (m + 1)],
                            wk_sb[:, 2 * kp:2 * kp + 2, :],
                            start=(kp == 0 and first[mi]),
                            stop=(kp == NK // 2 - 1), perf_mode=DR)
                kst[g]["pss"] = pss

            def k_B(g):      # scalar evac -> kp [128, 2x512]
                ps0, ps1 = kst[g]["pss"]
                kp = tmp("kp")
                nc.scalar.activation(kp[:, 0:512], ps0[:], AF.Copy,
                                     scale=kdescale)
                nc.scalar.activation(kp[:, 512:1024], ps1[:], AF.Copy,
                                     scale=kdescale)
                kst[g]["kp"] = kp

            def k_C(g):      # t1 = kp*cos (V 2x, full table); t2 half-swaps
                kp = kst[g]["kp"]
                t1 = tmp("t1k")
                nc.vector.tensor_tensor(
                    t1.rearrange("p (m c) -> p m c", m=2)[:],
                    kp.rearrange("p (m c) -> p m c", m=2)[:],
                    cost_sb[:, 2 * g:2 * g + 2, :], ALU.mult)
                t2 = tmp("t2k")
                for mi in range(2):
                    m = 2 * g + mi
                    k8 = kp[:, 512 * mi:512 * (mi + 1)] \
                        .rearrange("p (h s i) -> p h s i", h=NH, s=2, i=32)
                    t8 = t2[:, 512 * mi:512 * (mi + 1)] \
                        .rearrange("p (h s i) -> p h s i", h=NH, s=2, i=32)
                    sa = sint_sb[:, m, 0:32] \
                        .rearrange("p (a i) -> p a i", a=1) \
                        .broadcast_to([128, NH, 32])
                    sb_ = sint_sb[:, m, 32:64] \
                        .rearrange("p (a i) -> p a i", a=1) \
                        .broadcast_to([128, NH, 32])
                    nc.vector.tensor_tensor(t8[:, :, 0, :], k8[:, :, 1, :],
                                            sa[:], ALU.mult)
                    nc.vector.tensor_tensor(t8[:, :, 1, :], k8[:, :, 0, :],
                                            sb_[:], ALU.mult)
                kst[g]["t1"] = t1
                kst[g]["t2"] = t2

            def k_D(g):      # k2 = t1 + t2 (V 2x); k2m = min(k2,0) (V 4x)
                k2 = tmp("k2")
                nc.vector.tensor_tensor(k2[:], kst[g]["t1"][:], kst[g]["t2"][:],
                                        ALU.add)
                k2m = tmp("k2m")
                nc.vector.tensor_scalar_min(k2m[:], k2[:], 0.0)
                kst[g]["k2"] = k2
                kst[g]["k2m"] = k2m

            def k_E(g):      # ke = exp(k2m) (S)
                ke = tmp("ke")
                nc.scalar.activation(ke[:], kst[g]["k2m"][:], AF.Exp)
                kst[g]["ke"] = ke

            def k_F(g):      # kf = (k2 max 0) + ke  (one V stt pass)
                nc.vector.scalar_tensor_tensor(
                    kfp[g][:], kst[g]["k2"][:], 0.0, kst[g]["ke"][:],
                    ALU.max, ALU.add)
                kst[g].clear()

            with nc.named_scope("kproj"):
                for s in range(NKG + 5):
                    if s < NKG:
                        k_A(s)
                    if 0 <= s - 1 < NKG:
                        k_B(s - 1)
                    if 0 <= s - 4 < NKG:
                        k_F(s - 4)
                    if 0 <= s - 2 < NKG:
                        k_D(s - 2)
                    if 0 <= s - 3 < NKG:
                        k_E(s - 3)
                    if 0 <= s - 1 < NKG:
                        k_C(s - 1)

            # ---- V phase + kv accumulation ------------------------------
            wv_sb = sb.tile([128, NK, DQ], BF16, tag="w", bufs=1)
            mk_sb = sb.tile([128, NT], F32, tag="mask")
            nc.gpsimd.dma_start(mk_sb[:], maskc[:])
            wo_sb = sb.tile([128, NJ, D_MODEL], BF16, tag="wo")
            esel = sb.tile([128, 128], BF16, tag="esel")
            xv = [sb.tile([128, T], BF16, tag="xt", bufs=8, name=f"xv{k}")
                  for k in range(NK)]
            nc.gpsimd.dma_start(wv_sb[:], wv.rearrange("(k p) c -> p k c", p=128))
            for k in range(NK):
                nc.sync.dma_start(xv[k][:], xvt_d[128 * k:128 * (k + 1), :])
            nc.gpsimd.dma_start(wo_sb[:], wo.rearrange("(k p) c -> p k c", p=128))
            nc.gpsimd.dma_start(esel[:], eselp[:])
            if with_bv:
                bv_sb = sb.tile([1, DQ], BF16, tag="brow", bufs=3)
                nc.sync.dma_start(bv_sb[:], bv[:])

            kvp_t = ps.tile([128, 512], F32, tag="kv", bufs=1, name="kvp")
            kvp = kvp_t[:, 0:272]
            # open the kv accumulation group: zero the whole region so later
            # disjoint-region matmuls (start=False) all accumulate onto it
            nc.tensor.matmul(kvp[:], zrow[:, 0:128], zrow[:, 0:272],
                             start=True, stop=False)
            vst = [dict() for _ in range(NT)]

            def v_A(m):
                psv = ps.tile([128, 512], F32, tag="aux", bufs=2, name="psv")
                first = True
                if with_bv:
                    nc.tensor.matmul(psv[:], ones[:, 0:128], bv_sb[:],
                                     start=True, stop=False)
                    first = False
                for k in range(NK):
                    nc.tensor.matmul(
                        psv[:], xv[k][:, 128 * m:128 * (m + 1)],
                        wv_sb[:, k, :], start=first, stop=(k == NK - 1))
                    first = False
                vst[m]["psv"] = psv

            def v_B(m):      # scalar evac with fused mask scale; V mask col
                psv = vst[m]["psv"]
                v2 = sb.tile([128, NH, 68], BF16, tag="vv", bufs=4, name="v2")
                nc.scalar.activation(
                    v2[:, :, 0:64], psv.rearrange("p (h i) -> p h i", h=NH),
                    AF.Copy, scale=mk_sb[:, m:m + 1])
                nc.vector.tensor_copy(
                    v2[:, :, 64:65],
                    mk_sb[:, m:m + 1].rearrange("p (a i) -> p a i", a=1)
                    .broadcast_to([128, NH, 1]))
                vst[m]["v2"] = v2

            def v_C(m):      # kv' accumulation
                v2 = vst[m]["v2"]
                for h in range(NH):
                    r0 = 64 * (h % 2)
                    c0 = 68 * (h // 2)
                    nc.tensor.matmul(
                        kvp[r0:r0 + 64, c0:c0 + 68],
                        kfp[m // 2][:, 512 * (m % 2) + 64 * h:
                                     512 * (m % 2) + 64 * (h + 1)],
                        v2[:, h, :],
                        start=False, stop=False,
                        tile_position=(0, r0))
                vst[m].clear()

            with nc.named_scope("vproj"):
                for s in range(NT + 1):
                    if s < NT:
                        v_A(s)
                    if 0 <= s - 1 < NT:
                        v_B(s - 1)
                        v_C(s - 1)
            # close the kv group (single dep covering all kv matmuls)
            nc.tensor.matmul(kvp[:], zrow[:, 0:128], zrow[:, 0:272],
                             start=False, stop=True)

            # repack kv' into per-j block-diagonal [128,128] + ksum pack [128,8]
            kvblk = [sb.tile([128, 128], BF16, tag="kvb", bufs=NJ, name=f"kvb{j}")
                     for j in range(NJ)]
            kspack = sb.tile([128, 8], BF16, tag="ksp")
            nc.vector.memset(kspack[:], 0.0)
            for j in range(NJ):
                nc.vector.memset(kvblk[j][:], 0.0)
                nc.vector.tensor_copy(kvblk[j][0:64, 0:64],
                                      kvp[0:64, 68 * j:68 * j + 64])
                nc.vector.tensor_copy(kvblk[j][64:128, 64:128],
                                      kvp[64:128, 68 * j:68 * j + 64])
                nc.vector.tensor_copy(kspack[0:64, 2 * j:2 * j + 1],
                                      kvp[0:64, 68 * j + 64:68 * j + 65])
                nc.vector.tensor_copy(kspack[64:128, 2 * j + 1:2 * j + 2],
                                      kvp[64:128, 68 * j + 64:68 * j + 65])

            # ---- attention (feature-major) + output projection ----------
            ast = [dict() for _ in range(NC_)]

            def a_den(c):
                ch = slice(512 * c, 512 * (c + 1))
                den = ps.tile([128, 512], F32, tag="aux", bufs=2, name="den")
                for j in range(NJ):
                    nc.tensor.matmul(den[32 * j:32 * j + 2, :],
                                     kspack[:, 2 * j:2 * j + 2],
                                     qf_all[:, j, ch], start=True, stop=True,
                                     tile_position=(0, 32 * j))
                zrecf = sb.tile([128, 512], F32, tag="zrf", bufs=2, name="zrecf")
                with nc.allow_low_precision(reason="z scale approx is fine"):
                    nc.vector.reciprocal_approx_fast(zrecf[:], den[:])
                zrec = sb.tile([128, 512], BF16, tag="zr", bufs=2, name="zrec")
                nc.vector.tensor_copy(zrec[:], zrecf[:])
                ast[c]["zrec"] = zrec

            def a_attn(c):
                ch = slice(512 * c, 512 * (c + 1))
                zrec = ast[c]["zrec"]
                osbs = []
                for j in range(NJ):
                    zep = ps.tile([128, 512], F32, tag="mm", bufs=5, name="zep")
                    nc.tensor.matmul(zep[:], esel[32 * j:32 * j + 2, :],
                                     zrec[32 * j:32 * j + 2, :],
                                     start=True, stop=True,
                                     tile_position=(32 * j, 0))
                    zes = sb.tile([128, 512], BF16, tag="ze", bufs=4,
                                  name="zes")
                    nc.scalar.copy(zes[:], zep[:])
                    opp = ps.tile([128, 512], F32, tag="mm", bufs=5, name="opp")
                    nc.tensor.matmul(opp[:], kvblk[j][:], qf_all[:, j, ch],
                                     start=True, stop=True)
                    osb = sb.tile([128, 512], BF16, tag="osb", bufs=8,
                                  name=f"osb{j}")
                    nc.vector.tensor_tensor(osb[:], opp[:], zes[:], ALU.mult)
                    osbs.append(osb)
                ast[c]["osbs"] = osbs

            def a_y(c):
                osbs = ast[c]["osbs"]
                for mm_ in range(4):
                    m = 4 * c + mm_
                    yps = [ps.tile([128, 512], F32, tag="mm", bufs=5, name="yp")
                           for _ in range(2)]
                    for j in range(NJ):
                        for c2 in range(2):
                            nc.tensor.matmul(
                                yps[c2][:],
                                osbs[j][:, 128 * mm_:128 * (mm_ + 1)],
                                wo_sb[:, j, 512 * c2:512 * (c2 + 1)],
                                start=(j == 0), stop=(j == NJ - 1))
                    ysb = sb.tile([128, 1024], BF16, tag="ysb", bufs=3,
                                  name="ysb")
                    nc.scalar.copy(ysb[:, 0:512], yps[0][:])
                    if mm_ == 3:
                        nc.vector.tensor_copy(ysb[:, 512:1024], yps[1][:])
                    else:
                        nc.scalar.copy(ysb[:, 512:1024], yps[1][:])
                    nc.sync.dma_start(y[128 * m:128 * (m + 1), :], ysb[:])
                ast[c].clear()

            with nc.named_scope("attn"):
                for s in range(NC_ + 1):
                    if s < NC_:
                        a_den(s)
                    if 0 <= s - 1 < NC_:
                        a_attn(s - 1)
                        a_y(s - 1)

    nc.compile()
    return nc


def _host_prep(queries, keys, values, key_lengths, Wq, bq, Wk, bk, Wv, bv, Wo):
    """Build the per-core input maps (host side: transpose, cast, tables)."""
    B = queries.shape[0]
    # per-head [evens|odds] feature permutation
    pat = np.concatenate([np.arange(0, HEAD_DIM, 2), np.arange(1, HEAD_DIM, 2)])
    perm = np.concatenate([h * HEAD_DIM + pat for h in range(NH)])  # within 512

    inv_freq = 1.0 / (ROPE_THETA ** (np.arange(0, HEAD_DIM, 2, dtype=np.float64)
                                     / HEAD_DIM))  # [32]
    t = np.arange(T, dtype=np.float64)
    ang = t[:, None] * inv_freq[None, :]           # [T, 32]
    cos32 = np.cos(ang).astype(np.float32)
    sin32 = np.sin(ang).astype(np.float32)

    # feature-major tables [128, T]: row r: block = r % 64; i = block % 32
    idx = np.arange(128) % HEAD_DIM
    fidx = np.where(idx < 32, idx, idx - 32)
    cosfm = cos32[:, fidx].T.astype(BF)            # [128, T]
    sinfm = sin32[:, fidx].T.astype(BF)

    # token-major full cos table [128, NT*DQ] (per m: [128 tok, 512 feat]);
    # small sin table [128, NT*64]: [0:32]=-sin, [32:64]=+sin per token
    cidx = np.arange(DQ) % HEAD_DIM
    cf = np.where(cidx < 32, cidx, cidx - 32)
    costm_full = cos32[:, cf]                      # [T, 512]
    costm = np.ascontiguousarray(
        costm_full.reshape(NT, 128, DQ).transpose(1, 0, 2)
        .reshape(128, NT * DQ)).astype(BF)
    sin_t = sin32.reshape(NT, 128, 32).transpose(1, 0, 2)    # [128, NT, 32]
    sintsm = np.concatenate([-sin_t, sin_t], axis=2)         # [128, NT, 64]
    sintsm = np.ascontiguousarray(sintsm.reshape(128, NT * 64)).astype(BF)

    # rotation matrix R (permuted layout), lhsT = R.T
    R = np.zeros((128, 128), np.float32)
    for base in (0, 64):
        R[base + 0:base + 32, base + 32:base + 64] = -np.eye(32)
        R[base + 32:base + 64, base + 0:base + 32] = np.eye(32)
    rt = np.ascontiguousarray(R.T).astype(BF)
    esel = np.zeros((128, 128), np.float32)
    for j in range(NJ):
        esel[32 * j, 0:64] = 1.0
        esel[32 * j + 1, 64:128] = 1.0
    esel = esel.astype(BF)

    with_bq = bool(np.any(np.asarray(bq)))
    with_bk = bool(np.any(np.asarray(bk)))
    with_bv = bool(np.any(np.asarray(bv)))

    E4 = ml_dtypes.float8_e4m3fn
    q_np = np.asarray(queries, np.float32)
    k_np = np.asarray(keys, np.float32)
    sxq = 224.0 / max(float(np.abs(q_np).max()), 1e-30)
    sxk = 224.0 / max(float(np.abs(k_np).max()), 1e-30)
    swq = 224.0 / max(float(np.abs(np.asarray(Wq)).max()), 1e-30)
    swk = 224.0 / max(float(np.abs(np.asarray(Wk)).max()), 1e-30)
    xqt_b = [np.ascontiguousarray((q_np[b] * sxq).astype(E4).T)
             for b in range(B)]
    xkt_b = [np.ascontiguousarray((k_np[b] * sxk).astype(E4).T)
             for b in range(B)]
    xvt_b = [np.ascontiguousarray(np.asarray(values[b]).astype(BF).T)
             for b in range(B)]
    Wq = np.asarray(Wq, np.float32); Wk = np.asarray(Wk, np.float32)
    Wv = np.asarray(Wv, np.float32); Wo = np.asarray(Wo, np.float32)
    bq = np.asarray(bq, np.float32); bk = np.asarray(bk, np.float32)
    bv = np.asarray(bv, np.float32)

    in_maps = []
    for core in range(2 * B):
        b, g = core // 2, core % 2
        sl = slice(DQ * g, DQ * (g + 1))
        mask = (np.arange(T) < int(key_lengths[b])).astype(np.float32)
        maskc = np.ascontiguousarray(mask.reshape(NT, 128).T)
        m = {
            "xqt": xqt_b[b], "xkt": xkt_b[b], "xvt": xvt_b[b],
            "wq": np.ascontiguousarray(Wq[:, sl][:, perm] * swq).astype(E4),
            "wk": np.ascontiguousarray(Wk[:, sl][:, perm] * swk).astype(E4),
            "wv": np.ascontiguousarray(Wv[:, sl]).astype(BF),
            "wo": np.ascontiguousarray(Wo[sl, :]).astype(BF),
            "cosfm": cosfm, "sinfm": sinfm,
            "costm": costm, "sintsm": sintsm,
            "rt": rt, "esel": esel, "maskc": maskc,
        }
        if with_bq:
            m["bq"] = bq[sl][perm].reshape(1, DQ).astype(BF)
        if with_bk:
            m["bk"] = bk[sl][perm].reshape(1, DQ).astype(BF)
        if with_bv:
            m["bv"] = bv[sl].reshape(1, DQ).astype(BF)
        in_maps.append(m)
    descales = (1.0 / (sxq * swq), 1.0 / (sxk * swk))
    return in_maps, (with_bq, with_bk, with_bv), descales


def kernel(queries, keys, values, attn_mask, query_lengths, key_lengths,
           Wq, bq, Wk, bk, Wv, bv, Wo, bo):
    global LAST_RESULTS
    B = queries.shape[0]
    in_maps, bias_flags, descales = _host_prep(queries, keys, values,
                                                key_lengths, Wq, bq, Wk, bk,
                                                Wv, bv, Wo)
    nc = _build_program(*bias_flags, *descales)
    res = run_bass_kernel_spmd(nc, in_maps, core_ids=list(range(2 * B)))
    LAST_RESULTS = res
    bo = np.asarray(bo, np.float32)
    out = np.zeros((B, T, D_MODEL), np.float32)
    for b in range(B):
        if int(key_lengths[b]) == 0:
            # kv/ksum are all-zero; reference output is exactly bo
            out[b] = bo[None, :]
        else:
            out[b] = (np.asarray(res.results[2 * b]["y"], np.float32)
                      + np.asarray(res.results[2 * b + 1]["y"], np.float32)
                      + bo)
    return out
